# revision 1
# baseline (speedup 1.0000x reference)
"""Gated multi-head self-attention on 8 Trainium2 NeuronCores.

Sharding: batch (B=2) x head-groups (4 groups of 4 heads) -> 8 cores.
Each core computes, for its batch b and its 4 heads:
    partial_out[t, e] = sum_h gate[h] * (softmax(Q_h K_h^T / 8) (V_h + bv_h) Wo_h + bo_h)
The host sums the 4 head-group partials per batch (the "all-reduce") and
stacks the two batches.

Per-core dataflow (all matmuls in float32r = full-rate fp32, ~1.5e-4 rel):
  hT [E, T] (host-transposed)  --PE-->  QT/KT [128(2x64d), T] per head-pair
                               --PE-->  V [T, 256(4x64d)] (+bias via K=1 matmul)
  scoresT[s,t] = K^T Q per head (row-tiled pairs)  --ACT--> exp (bf16)
  rowsum via DVE chain-add + ones-matmul; PV col-tiled pairs -> ctxT
  ctxT/rowsum (DVE) --PE row-tiled--> out[t, e] += bias (K=1 matmul)
"""

import numpy as np
import ml_dtypes
from contextlib import ExitStack

import concourse.bass as bass
import concourse.tile as tile
from concourse import bacc, mybir
from concourse import bass_utils

E, H, D = 1024, 16, 64
B, T = 2, 2048
NCORES = 8
P = 128
TC = 512          # t-chunk (PSUM bank = 512 fp32)
NTC = T // TC     # 4 t-chunks
NST = T // P      # 16 s-tiles
NEC = E // P      # 8 e-chunks

F32 = mybir.dt.float32
F32R = mybir.dt.float32r
BF16 = mybir.dt.bfloat16


def build_kernel():
    nc = bacc.Bacc("TRN2", target_bir_lowering=False, debug=False,
                   num_devices=NCORES)
    hT = nc.dram_tensor("hT", [NEC, P, T], F32R, kind="ExternalInput").ap()
    wq = nc.dram_tensor("wq", [2, P, NEC, P], F32R, kind="ExternalInput").ap()
    wk = nc.dram_tensor("wk", [2, P, NEC, P], F32R, kind="ExternalInput").ap()
    wv = nc.dram_tensor("wv", [P, NEC, 256], F32R, kind="ExternalInput").ap()
    wo = nc.dram_tensor("wo", [2, P, E], F32R, kind="ExternalInput").ap()
    bq = nc.dram_tensor("bq", [2, 1, P], F32R, kind="ExternalInput").ap()
    bk = nc.dram_tensor("bk", [2, 1, P], F32R, kind="ExternalInput").ap()
    bv = nc.dram_tensor("bv", [1, 256], F32R, kind="ExternalInput").ap()
    bo = nc.dram_tensor("bo", [1, E], F32R, kind="ExternalInput").ap()
    ones_r = nc.dram_tensor("ones_r", [1, TC], F32R, kind="ExternalInput").ap()
    ones_b = nc.dram_tensor("ones_b", [P, 1], BF16, kind="ExternalInput").ap()
    sel = nc.dram_tensor("sel", [1, 2 * P], F32R, kind="ExternalInput").ap()
    out = nc.dram_tensor("out", [T, E], F32, kind="ExternalOutput").ap()

    with tile.TileContext(nc) as tc:
        with ExitStack() as ctx:
            persist = ctx.enter_context(tc.tile_pool(name="persist", bufs=1))
            work = ctx.enter_context(tc.tile_pool(name="work", bufs=4))
            rspool = ctx.enter_context(tc.tile_pool(name="rspool", bufs=2))
            ps_s = ctx.enter_context(tc.tile_pool(name="ps_s", bufs=2, space="PSUM"))
            ps_ctx = ctx.enter_context(tc.tile_pool(name="ps_ctx", bufs=2, space="PSUM"))
            ps_misc = ctx.enter_context(tc.tile_pool(name="ps_misc", bufs=2, space="PSUM"))

            # ---- persistent SBUF tensors ----
            hT_sb = persist.tile([P, NEC, T], F32R, tag="hT")
            wq_sb = persist.tile([P, 2, NEC, P], F32R, tag="wq")
            wk_sb = persist.tile([P, 2, NEC, P], F32R, tag="wk")
            wv_sb = persist.tile([P, NEC, 256], F32R, tag="wv")
            wo_sb = persist.tile([P, 2, E], F32R, tag="wo")
            bq_sb = persist.tile([1, 2, P], F32R, tag="bq")
            bk_sb = persist.tile([1, 2, P], F32R, tag="bk")
            bv_sb = persist.tile([1, 256], F32R, tag="bv")
            bo_sb = persist.tile([1, E], F32R, tag="bo")
            on_r = persist.tile([1, TC], F32R, tag="on_r")
            on_b = persist.tile([P, 1], BF16, tag="on_b")
            sel_sb = persist.tile([1, 2 * P], F32R, tag="sel")
            QT_sb = persist.tile([P, 2, T], F32R, tag="QT")
            KT_sb = persist.tile([P, 2, T], F32R, tag="KT")
            V_sb = persist.tile([P, NST, 256], BF16, tag="V")
            ctx_sb = persist.tile([P, 2, T], F32R, tag="ctx")

            with nc.named_scope("load"):
                for ec in range(NEC):
                    nc.sync.dma_start(hT_sb[:, ec, :], hT[ec])
                nc.sync.dma_start(wq_sb[:], wq.rearrange("a p c d -> p a c d"))
                nc.sync.dma_start(wk_sb[:], wk.rearrange("a p c d -> p a c d"))
                nc.sync.dma_start(wv_sb[:], wv)
                nc.sync.dma_start(wo_sb[:], wo.rearrange("a p e -> p a e"))
                nc.sync.dma_start(bq_sb[:], bq.rearrange("a o p -> o a p"))
                nc.sync.dma_start(bk_sb[:], bk.rearrange("a o p -> o a p"))
                nc.sync.dma_start(bv_sb[:], bv)
                nc.sync.dma_start(bo_sb[:], bo)
                nc.sync.dma_start(on_r[:], ones_r)
                nc.sync.dma_start(on_b[:], ones_b)
                nc.sync.dma_start(sel_sb[:], sel)

            # ---- phase 1: QKV projections ----
            with nc.named_scope("qkv"):
                for pr in range(2):
                    for (w_sb, b_sb, dst) in ((wq_sb, bq_sb, QT_sb), (wk_sb, bk_sb, KT_sb)):
                        for tch in range(NTC):
                            ps = ps_misc.tile([P, TC], F32, tag="ps_misc")
                            for ec in range(NEC):
                                nc.tensor.matmul(
                                    ps[:], w_sb[:, pr, ec, :],
                                    hT_sb[:, ec, tch * TC:(tch + 1) * TC],
                                    start=(ec == 0), stop=False)
                            nc.tensor.matmul(ps[:], b_sb[:, pr, :], on_r[:],
                                             start=False, stop=True)
                            nc.vector.tensor_copy(
                                dst[:, pr, tch * TC:(tch + 1) * TC], ps[:])
                for st in range(NST):
                    ps = ps_misc.tile([P, TC], F32, tag="ps_misc")
                    psv = ps[:, :256]
                    for ec in range(NEC):
                        nc.tensor.matmul(
                            psv, hT_sb[:, ec, st * P:(st + 1) * P],
                            wv_sb[:, ec, :], start=(ec == 0), stop=False)
                    nc.tensor.matmul(psv, on_r[:1, :P], bv_sb[:],
                                     start=False, stop=True)
                    nc.vector.tensor_copy(V_sb[:, st, :], psv)

            # ---- phase 2: attention ----
            with nc.named_scope("attn"):
                for tch in range(NTC):
                    t0 = tch * TC
                    for pr in range(2):
                        pctx = ps_ctx.tile([P, TC], F32, tag="ps_ctx")
                        rs = rspool.tile([P, 2 * TC], BF16, tag="rs")
                        for st in range(NST):
                            s0 = st * P
                            pss = ps_s.tile([P, 2 * TC], F32, tag="ps_s")
                            nc.tensor.matmul(
                                pss[:, :TC], KT_sb[0:64, pr, s0:s0 + P],
                                QT_sb[0:64, pr, t0:t0 + TC],
                                start=True, stop=True, tile_position=(0, 0))
                            nc.tensor.matmul(
                                pss[:, TC:], KT_sb[64:P, pr, s0:s0 + P],
                                QT_sb[64:P, pr, t0:t0 + TC],
                                start=True, stop=True, tile_position=(64, 0))
                            ex = work.tile([P, 2 * TC], BF16, tag="expT")
                            nc.scalar.activation(
                                ex[:], pss[:],
                                mybir.ActivationFunctionType.Exp, scale=0.125)
                            if st == 0:
                                nc.vector.tensor_copy(rs[:], ex[:])
                            else:
                                nc.vector.tensor_add(rs[:], rs[:], ex[:])
                            c0 = pr * P
                            nc.tensor.matmul(
                                pctx[0:64, :], V_sb[:, st, c0:c0 + 64],
                                ex[:, :TC],
                                start=(st == 0), stop=(st == NST - 1),
                                tile_position=(0, 0), skip_group_check=True)
                            nc.tensor.matmul(
                                pctx[64:P, :], V_sb[:, st, c0 + 64:c0 + P],
                                ex[:, TC:],
                                start=(st == 0), stop=(st == NST - 1),
                                tile_position=(0, 64), skip_group_check=True)
                        # rowsums -> reciprocals
                        rcps = []
                        for hh in range(2):
                            prs = ps_misc.tile([P, TC], F32, tag="ps_misc")
                            nc.tensor.matmul(prs[0:1, :], on_b[:],
                                             rs[:, hh * TC:(hh + 1) * TC],
                                             start=True, stop=True)
                            rcp = work.tile([1, TC], F32R, tag="rcp")
                            with nc.allow_low_precision(reason="f32r reciprocal is plenty for softmax denom"):
                                nc.vector.reciprocal(rcp[:], prs[0:1, :])
                            rcps.append(rcp)
                        pR = ps_misc.tile([P, TC], F32, tag="ps_misc")
                        nc.tensor.matmul(pR[:], sel_sb[:, 0:P], rcps[0][:],
                                         start=True, stop=False)
                        nc.tensor.matmul(pR[:], sel_sb[:, P:2 * P], rcps[1][:],
                                         start=False, stop=True)
                        R_sb = work.tile([P, TC], F32R, tag="R")
                        nc.vector.tensor_copy(R_sb[:], pR[:])
                        nc.vector.tensor_tensor(
                            ctx_sb[:, pr, t0:t0 + TC], pctx[:], R_sb[:],
                            mybir.AluOpType.mult)

            # ---- phase 3: output projection ----
            with nc.named_scope("outproj"):
                for tt in range(NST):
                    for ec2 in range(2):
                        pso = ps_misc.tile([P, TC], F32, tag="ps_misc")
                        for pr in range(2):
                            nc.tensor.matmul(
                                pso[:], ctx_sb[:, pr, tt * P:(tt + 1) * P],
                                wo_sb[:, pr, ec2 * TC:(ec2 + 1) * TC],
                                start=(pr == 0), stop=False)
                        nc.tensor.matmul(pso[:], on_r[:1, :P],
                                         bo_sb[:, ec2 * TC:(ec2 + 1) * TC],
                                         start=False, stop=True)
                        o_sb = work.tile([P, TC], F32, tag="o")
                        nc.vector.tensor_copy(o_sb[:], pso[:])
                        nc.sync.dma_start(
                            out[tt * P:(tt + 1) * P, ec2 * TC:(ec2 + 1) * TC],
                            o_sb[:])
    nc.compile()
    return nc


_NC = None


def _get_nc():
    global _NC
    if _NC is None:
        _NC = build_kernel()
    return _NC


def make_in_maps(hidden_states, Wq, bq, Wk, bk, Wv, bv, Wo, bo, gate):
    f = np.float32
    hidden_states = np.asarray(hidden_states, f)
    Wq, bq = np.asarray(Wq, f), np.asarray(bq, f)
    Wk, bk = np.asarray(Wk, f), np.asarray(bk, f)
    Wv, bv = np.asarray(Wv, f), np.asarray(bv, f)
    Wo, bo = np.asarray(Wo, f), np.asarray(bo, f)
    gate = np.asarray(gate, f)

    hT_b = [np.ascontiguousarray(hidden_states[b].T).reshape(NEC, P, T)
            for b in range(B)]
    ones_r = np.ones((1, TC), f)
    ones_b = np.ones((P, 1), ml_dtypes.bfloat16)
    sel_np = np.zeros((1, 2 * P), f)
    sel_np[0, 0:64] = 1.0      # head-A rows of R
    sel_np[0, P + 64:2 * P] = 1.0  # head-B rows of R

    in_maps = []
    for core in range(NCORES):
        b, hg = divmod(core, 4)
        hs = [4 * hg + i for i in range(4)]
        # [2, 128, NEC, 128]: per pair, (e_in, e_chunk, d-packed)
        def pack_qk(W):
            outw = np.empty((2, P, NEC, P), f)
            for pr in range(2):
                pair = np.concatenate(
                    [W[hs[2 * pr]], W[hs[2 * pr + 1]]], axis=1)  # [E, 128]
                outw[pr] = pair.reshape(NEC, P, P).transpose(1, 0, 2)
            return outw
        wv_np = np.concatenate([Wv[h] for h in hs], axis=1)  # [E, 256]
        wv_np = wv_np.reshape(NEC, P, 256).transpose(1, 0, 2)
        wo_np = np.empty((2, P, E), f)
        bq_np = np.empty((2, 1, P), f)
        bk_np = np.empty((2, 1, P), f)
        for pr in range(2):
            h0, h1 = hs[2 * pr], hs[2 * pr + 1]
            wo_np[pr] = np.concatenate(
                [gate[h0] * Wo[h0], gate[h1] * Wo[h1]], axis=0)  # [128, E]
            bq_np[pr, 0] = np.concatenate([bq[h0], bq[h1]])
            bk_np[pr, 0] = np.concatenate([bk[h0], bk[h1]])
        bv_np = np.concatenate([bv[h] for h in hs])[None, :]  # [1, 256]
        bo_np = sum(gate[h] * bo[h] for h in hs)[None, :]     # [1, E]
        in_maps.append(dict(
            hT=np.ascontiguousarray(hT_b[b]),
            wq=np.ascontiguousarray(pack_qk(Wq)),
            wk=np.ascontiguousarray(pack_qk(Wk)),
            wv=np.ascontiguousarray(wv_np),
            wo=np.ascontiguousarray(wo_np),
            bq=bq_np, bk=bk_np,
            bv=np.ascontiguousarray(bv_np),
            bo=np.ascontiguousarray(bo_np),
            ones_r=ones_r, ones_b=ones_b, sel=sel_np,
        ))
    return in_maps


def kernel(hidden_states, Wq, bq, Wk, bk, Wv, bv, Wo, bo, gate, _trace=False,
           **run_kwargs):
    nc = _get_nc()
    in_maps = make_in_maps(hidden_states, Wq, bq, Wk, bk, Wv, bv, Wo, bo, gate)
    res = bass_utils.run_bass_kernel_spmd(
        nc, in_maps, core_ids=list(range(NCORES)), trace=_trace, **run_kwargs)
    outs = [r["out"] for r in res.results]
    full = np.stack([
        outs[0] + outs[1] + outs[2] + outs[3],
        outs[4] + outs[5] + outs[6] + outs[7],
    ]).astype(np.float32)
    kernel.last_result = res
    return full



# revision 2
# speedup vs baseline: 1.7129x; 1.7129x over previous
"""Gated multi-head self-attention on 8 Trainium2 NeuronCores.

Sharding: batch (B=2) x head-groups (4 groups of 4 heads) -> 8 cores.
Each core computes, for its batch b and its 4 heads:
    partial_out[t, e] = sum_h gate[h] * (softmax(Q_h K_h^T / 8) (V_h + bv_h) Wo_h)
The host sums the 4 head-group partials per batch, adds sum_h gate_h*bo_h,
and stacks the two batches.

All matmuls in bf16 (FWL-eligible stationaries, full-rate moving operands);
PSUM accumulation fp32.  Per-core dataflow:
  prologue: K = Wk^T h (+bk via ACT bias), Q(tch0); V (+bv via K=1 matmul)
            merged into the first attention block.
  per (t-chunk, head-pair): scoresT[s,t] = K^T Q row-tiled pairs -> exp (ACT,
  bf16) -> rowsum ping-pong adds (DVE) + PV col-tiled pairs (PE, PSUM accum);
  rowsum broadcast via mask-matmul; 1/x via DVE reciprocal_approx_fast;
  ctxT = pctx * R (DVE, bf16).  Outproj: ctxT^T Wo per 128-row tile (PE),
  DVE copy, DMA out in bf16.
"""

import numpy as np
import ml_dtypes
from contextlib import ExitStack

import concourse.bass as bass
import concourse.tile as tile
from concourse import bacc, mybir
from concourse import bass_utils

E, H, D = 1024, 16, 64
B, T = 2, 2048
NCORES = 8
P = 128
TC = 512          # t-chunk (PSUM bank = 512 fp32)
NTC = T // TC     # 4 t-chunks
NST = T // P      # 16 s-tiles
NEC = E // P      # 8 e-chunks

F32 = mybir.dt.float32
BF16 = mybir.dt.bfloat16
BF = ml_dtypes.bfloat16


def build_kernel():
    nc = bacc.Bacc("TRN2", target_bir_lowering=False, debug=False,
                   num_devices=NCORES)
    hT = nc.dram_tensor("hT", [P, NEC, T], BF16, kind="ExternalInput").ap()
    wq = nc.dram_tensor("wq", [P, 2, NEC, P], BF16, kind="ExternalInput").ap()
    wk = nc.dram_tensor("wk", [P, 2, NEC, P], BF16, kind="ExternalInput").ap()
    wv = nc.dram_tensor("wv", [P, NEC, 256], BF16, kind="ExternalInput").ap()
    wo = nc.dram_tensor("wo", [P, 2, E], BF16, kind="ExternalInput").ap()
    bq = nc.dram_tensor("bq", [P, 2], F32, kind="ExternalInput").ap()
    bk = nc.dram_tensor("bk", [P, 2], F32, kind="ExternalInput").ap()
    bv = nc.dram_tensor("bv", [1, 256], BF16, kind="ExternalInput").ap()
    ones_r = nc.dram_tensor("ones_r", [1, P], BF16, kind="ExternalInput").ap()
    mask = nc.dram_tensor("mask", [P, 2 * P], BF16, kind="ExternalInput").ap()
    out = nc.dram_tensor("out", [T, E], BF16, kind="ExternalOutput").ap()

    with tile.TileContext(nc) as tc:
        with ExitStack() as ctx:
            persist = ctx.enter_context(tc.tile_pool(name="persist", bufs=1))
            work = ctx.enter_context(tc.tile_pool(name="work", bufs=4))
            rspool = ctx.enter_context(tc.tile_pool(name="rspool", bufs=4))
            rpool = ctx.enter_context(tc.tile_pool(name="rpool", bufs=2))
            opool = ctx.enter_context(tc.tile_pool(name="opool", bufs=3))
            ps_s = ctx.enter_context(tc.tile_pool(name="ps_s", bufs=2, space="PSUM"))
            ps_ctx = ctx.enter_context(tc.tile_pool(name="ps_ctx", bufs=2, space="PSUM"))
            ps_misc = ctx.enter_context(tc.tile_pool(name="ps_misc", bufs=2, space="PSUM"))

            # ---- persistent SBUF tensors ----
            hT_sb = persist.tile([P, NEC, T], BF16, tag="hT")
            wq_sb = persist.tile([P, 2, NEC, P], BF16, tag="wq")
            wk_sb = persist.tile([P, 2, NEC, P], BF16, tag="wk")
            wv_sb = persist.tile([P, NEC, 256], BF16, tag="wv")
            wo_sb = persist.tile([P, 2, E], BF16, tag="wo")
            bq_sb = persist.tile([P, 2], F32, tag="bq")
            bk_sb = persist.tile([P, 2], F32, tag="bk")
            bv_sb = persist.tile([1, 256], BF16, tag="bv")
            on_r = persist.tile([1, P], BF16, tag="on_r")
            mask_sb = persist.tile([P, 2 * P], BF16, tag="mask")
            QT_sb = persist.tile([P, 2, T], BF16, tag="QT")
            KT_sb = persist.tile([P, 2, T], BF16, tag="KT")
            V_sb = persist.tile([P, NST, 256], BF16, tag="V")
            ctx_sb = persist.tile([P, 2, T], BF16, tag="ctx")

            with nc.named_scope("load"):
                nc.sync.dma_start(wk_sb[:], wk)
                nc.sync.dma_start(bk_sb[:], bk)
                for ec in range(NEC):
                    nc.sync.dma_start(hT_sb[:, ec, :], hT[:, ec, :])
                nc.sync.dma_start(wq_sb[:], wq)
                nc.sync.dma_start(bq_sb[:], bq)
                nc.sync.dma_start(wv_sb[:], wv)
                nc.sync.dma_start(bv_sb[:], bv)
                nc.sync.dma_start(on_r[:], ones_r)
                nc.sync.dma_start(mask_sb[:], mask)
                nc.sync.dma_start(wo_sb[:], wo)

            def qk_proj(w_sb, b_sb, dst, pr, tch):
                t0 = tch * TC
                ps = ps_misc.tile([P, TC], F32, tag="ps_misc")
                for ec in range(NEC):
                    nc.tensor.matmul(ps[:], w_sb[:, pr, ec, :],
                                     hT_sb[:, ec, t0:t0 + TC],
                                     start=(ec == 0), stop=(ec == NEC - 1))
                nc.scalar.activation(dst[:, pr, t0:t0 + TC], ps[:],
                                     mybir.ActivationFunctionType.Identity,
                                     bias=b_sb[:, pr:pr + 1], scale=1.0)

            def v_proj(st):
                ps = ps_misc.tile([P, TC], F32, tag="ps_misc")
                psv = ps[:, :256]
                for ec in range(NEC):
                    nc.tensor.matmul(psv, hT_sb[:, ec, st * P:(st + 1) * P],
                                     wv_sb[:, ec, :], start=(ec == 0), stop=False)
                nc.tensor.matmul(psv, on_r[:], bv_sb[:], start=False, stop=True)
                nc.vector.tensor_copy(V_sb[:, st, :], psv)

            def attn_block(tch, pr, merge_v):
                t0 = tch * TC
                c0 = pr * P
                pctx = ps_ctx.tile([P, TC], F32, tag="ps_ctx")
                rs0 = rspool.tile([P, 2 * TC], BF16, tag="rs")
                rs1 = rspool.tile([P, 2 * TC], BF16, tag="rs")
                rstiles = (rs0, rs1)
                for st in range(NST):
                    if merge_v:
                        v_proj(st)
                    s0 = st * P
                    pss = ps_s.tile([P, 2 * TC], F32, tag="ps_s")
                    nc.tensor.matmul(
                        pss[:, :TC], KT_sb[0:64, pr, s0:s0 + P],
                        QT_sb[0:64, pr, t0:t0 + TC],
                        start=True, stop=True, tile_position=(0, 0))
                    nc.tensor.matmul(
                        pss[:, TC:], KT_sb[64:P, pr, s0:s0 + P],
                        QT_sb[64:P, pr, t0:t0 + TC],
                        start=True, stop=True, tile_position=(64, 0))
                    ex = work.tile([P, 2 * TC], BF16, tag="expT")
                    nc.scalar.activation(
                        ex[:], pss[:],
                        mybir.ActivationFunctionType.Exp, scale=0.125)
                    if st == 0:
                        nc.vector.tensor_copy(rs0[:], ex[:])
                    else:
                        nc.vector.tensor_add(rstiles[st % 2][:],
                                             rstiles[(st + 1) % 2][:], ex[:])
                    nc.tensor.matmul(
                        pctx[0:64, :], V_sb[:, st, c0:c0 + 64],
                        ex[:, :TC],
                        start=(st == 0), stop=(st == NST - 1),
                        tile_position=(0, 0), skip_group_check=True)
                    nc.tensor.matmul(
                        pctx[64:P, :], V_sb[:, st, c0 + 64:c0 + P],
                        ex[:, TC:],
                        start=(st == 0), stop=(st == NST - 1),
                        tile_position=(0, 64), skip_group_check=True)
                rs_fin = rstiles[(NST - 1) % 2]
                pR = ps_misc.tile([P, TC], F32, tag="ps_misc")
                nc.tensor.matmul(pR[:], mask_sb[:, 0:P], rs_fin[:, :TC],
                                 start=True, stop=False)
                nc.tensor.matmul(pR[:], mask_sb[:, P:2 * P], rs_fin[:, TC:],
                                 start=False, stop=True)
                R_sb = rpool.tile([P, TC], F32, tag="R")
                with nc.allow_low_precision(reason="~51-ULP recip is plenty for softmax denom"):
                    nc.vector.reciprocal_approx_fast(R_sb[:], pR[:])
                nc.vector.tensor_tensor(
                    ctx_sb[:, pr, t0:t0 + TC], pctx[:], R_sb[:],
                    mybir.AluOpType.mult)

            def outproj(tch):
                for tt in range(tch * 4, tch * 4 + 4):
                    for ec2 in range(2):
                        pso = ps_misc.tile([P, TC], F32, tag="ps_misc")
                        for pr in range(2):
                            nc.tensor.matmul(
                                pso[:], ctx_sb[:, pr, tt * P:(tt + 1) * P],
                                wo_sb[:, pr, ec2 * TC:(ec2 + 1) * TC],
                                start=(pr == 0), stop=(pr == 1))
                        o_sb = opool.tile([P, TC], BF16, tag="o")
                        nc.vector.tensor_copy(o_sb[:], pso[:])
                        nc.sync.dma_start(
                            out[tt * P:(tt + 1) * P, ec2 * TC:(ec2 + 1) * TC],
                            o_sb[:])

            with nc.named_scope("qkv"):
                for pr in range(2):
                    for tch in range(NTC):
                        qk_proj(wk_sb, bk_sb, KT_sb, pr, tch)
                for pr in range(2):
                    qk_proj(wq_sb, bq_sb, QT_sb, pr, 0)

            with nc.named_scope("attn"):
                for tch in range(NTC):
                    for pr in range(2):
                        attn_block(tch, pr, merge_v=(tch == 0 and pr == 0))
                    if tch < NTC - 1:
                        for pr in range(2):
                            qk_proj(wq_sb, bq_sb, QT_sb, pr, tch + 1)
                    with nc.named_scope("outproj"):
                        outproj(tch)
    nc.compile()
    return nc


_NC = None


def _get_nc():
    global _NC
    if _NC is None:
        _NC = build_kernel()
    return _NC


def make_in_maps(hidden_states, Wq, bq, Wk, bk, Wv, bv, Wo, bo, gate):
    f = np.float32
    hidden_states = np.asarray(hidden_states, f)
    Wq, bq = np.asarray(Wq, f), np.asarray(bq, f)
    Wk, bk = np.asarray(Wk, f), np.asarray(bk, f)
    Wv, bv = np.asarray(Wv, f), np.asarray(bv, f)
    Wo, bo = np.asarray(Wo, f), np.asarray(bo, f)
    gate = np.asarray(gate, f)

    # [P, NEC, T] bf16 per batch
    hT_b = [np.ascontiguousarray(
                hidden_states[b].T.reshape(NEC, P, T).transpose(1, 0, 2)
            ).astype(BF) for b in range(B)]
    ones_r = np.ones((1, P), BF)
    mask_np = np.zeros((P, 2 * P), f)
    mask_np[:, 0:64] = 1.0        # maskA: broadcast head-A rowsum to rows 0-63
    mask_np[:, P + 64:2 * P] = 1.0  # maskB: head-B rowsum to rows 64-127
    mask_np = mask_np.astype(BF)

    in_maps = []
    for core in range(NCORES):
        b, hg = divmod(core, 4)
        hs = [4 * hg + i for i in range(4)]

        def pack_qk(W):
            outw = np.empty((2, NEC, P, P), f)
            for pr in range(2):
                pair = np.concatenate(
                    [W[hs[2 * pr]], W[hs[2 * pr + 1]]], axis=1)  # [E, 128]
                outw[pr] = pair.reshape(NEC, P, P)
            # -> [P(e-part), 2, NEC, P(d-pair)]
            return np.ascontiguousarray(outw.transpose(2, 0, 1, 3)).astype(BF)

        def pack_b(bx):
            o = np.empty((P, 2), f)
            for pr in range(2):
                o[:, pr] = np.concatenate([bx[hs[2 * pr]], bx[hs[2 * pr + 1]]])
            return np.ascontiguousarray(o)

        wv_np = np.concatenate([Wv[h] for h in hs], axis=1)  # [E, 256]
        wv_np = np.ascontiguousarray(
            wv_np.reshape(NEC, P, 256).transpose(1, 0, 2)).astype(BF)
        wo_np = np.empty((2, P, E), f)
        for pr in range(2):
            h0, h1 = hs[2 * pr], hs[2 * pr + 1]
            wo_np[pr] = np.concatenate(
                [gate[h0] * Wo[h0], gate[h1] * Wo[h1]], axis=0)  # [128, E]
        wo_np = np.ascontiguousarray(wo_np.transpose(1, 0, 2)).astype(BF)
        bv_np = np.concatenate([bv[h] for h in hs])[None, :].astype(BF)
        in_maps.append(dict(
            hT=hT_b[b],
            wq=pack_qk(Wq), wk=pack_qk(Wk),
            wv=wv_np, wo=wo_np,
            bq=pack_b(bq), bk=pack_b(bk),
            bv=np.ascontiguousarray(bv_np),
            ones_r=ones_r, mask=mask_np,
        ))
    bo_sum = (gate[:, None] * bo).sum(axis=0).astype(f)  # [E]
    return in_maps, bo_sum


def kernel(hidden_states, Wq, bq, Wk, bk, Wv, bv, Wo, bo, gate, _trace=False,
           **run_kwargs):
    nc = _get_nc()
    in_maps, bo_sum = make_in_maps(
        hidden_states, Wq, bq, Wk, bk, Wv, bv, Wo, bo, gate)
    res = bass_utils.run_bass_kernel_spmd(
        nc, in_maps, core_ids=list(range(NCORES)), trace=_trace, **run_kwargs)
    outs = [np.asarray(r["out"], np.float32) for r in res.results]
    full = np.stack([
        outs[0] + outs[1] + outs[2] + outs[3] + bo_sum,
        outs[4] + outs[5] + outs[6] + outs[7] + bo_sum,
    ]).astype(np.float32)
    kernel.last_result = res
    return full
